# revision 11
# baseline (speedup 1.0000x reference)
"""ClusteringLoss (vq_codebook) Trainium2 kernel.

Data-parallel over N across 8 NeuronCores. Per core (N_loc = 32768):
  pass 1: s2 = 2*F@C^T (PE bf16, fp32 PSUM); score = c2 - s2 fused with the
          row-min reduce (DVE tensor_tensor_reduce); one-hot H = (score<=min);
          per-cluster [sums|counts] += H^T @ [F|1] (PE bf16, PSUM-accumulated)
  all-reduce [sums|counts] across the 8 cores (on-device collective)
  means = sums / max(counts,1) (fp32); mt2 = -2*means^T (bf16)
  pass 2: g2 = F@mt2 + m2_j (PE); gm_i = rowsum(H .* g2) (DVE fused);
          dev = sqrt(gm + ||f||^2) batched; per-cluster dev sums += H^T @ dev
  all-reduce dev sums; host does the final O(K) scalar reduction and the
  (tiny) K x K inter-cluster term.
"""

import numpy as np
import ml_dtypes

import concourse.bass as bass
import concourse.bacc as bacc
import concourse.tile as tile
from concourse import mybir
from concourse.bass_utils import run_bass_kernel_spmd

F32 = mybir.dt.float32
BF16 = mybir.dt.bfloat16
FP8 = mybir.dt.float8e4
ALU = mybir.AluOpType
ACTF = mybir.ActivationFunctionType

N, D, K = 262144, 128, 256
NCORES = 8
P = 128
MARGIN = 1.0
EPS = 0.01  # keeps sqrt's argument positive if a point coincides with its mean

# DMA chunk width (in 129-col fn tiles / 128-col ft tiles) per dma_start
TPC = 16  # tiles per DMA chunk

import os
_USE_TTR = os.environ.get("USE_TTR", "1") == "1"


def build_program(T, num_cores):
    """Build the SPMD Bass program. T = number of 128-row tiles per core."""
    nloc = T * P
    H2 = K // P  # number of 128-wide halves of the cluster axis (2)
    W = D + 1
    nc = bacc.Bacc(
        "TRN2",
        target_bir_lowering=False,
        debug=False,
        num_devices=num_cores,
    )

    ft_d = nc.dram_tensor("ft", [P, nloc], BF16, kind="ExternalInput").ap()
    fn_d = nc.dram_tensor("fn", [P, T * W], FP8, kind="ExternalInput").ap()
    f2_d = nc.dram_tensor("f2", [P, T], F32, kind="ExternalInput").ap()
    ct2_d = nc.dram_tensor("ct2", [P, K], BF16, kind="ExternalInput").ap()
    c2bc_d = nc.dram_tensor("c2bc", [P, K], F32, kind="ExternalInput").ap()
    ident_d = nc.dram_tensor("ident", [P, P], F32, kind="ExternalInput").ap()
    onesrow_d = nc.dram_tensor("onesrow", [1, P], BF16, kind="ExternalInput").ap()
    out_d = nc.dram_tensor("out", [P, 4], F32, kind="ExternalOutput").ap()

    from contextlib import ExitStack

    with tile.TileContext(nc) as tc, ExitStack() as ctx:
        const = ctx.enter_context(tc.tile_pool(name="const", bufs=1))
        big = ctx.enter_context(tc.tile_pool(name="big", bufs=1))
        dram = ctx.enter_context(tc.tile_pool(name="dram", bufs=1, space="DRAM"))

        ct2 = const.tile([P, K], BF16)
        nc.sync.dma_start(ct2[:], ct2_d)
        c2bc = const.tile([P, K], F32)
        nc.sync.dma_start(c2bc[:], c2bc_d)
        ident = const.tile([P, P], F32)
        nc.sync.dma_start(ident[:], ident_d)
        onesrow = const.tile([1, P], BF16)
        nc.sync.dma_start(onesrow[:], onesrow_d)
        f2_all = const.tile([P, T], F32)
        nc.sync.dma_start(f2_all[:], f2_d)

        # Resident feature shard, both layouts, loaded in chunks so compute
        # on early tiles overlaps later DMA.
        nchunks = (T + TPC - 1) // TPC
        ft_chunks, fn_chunks = [], []
        for c in range(nchunks):
            t0, t1 = c * TPC, min((c + 1) * TPC, T)
            ftc = big.tile([P, (t1 - t0) * P], BF16, tag=f"ftc{c}", name=f"ftc{c}")
            nc.sync.dma_start(ftc[:], ft_d[:, t0 * P : t1 * P])
            ft_chunks.append(ftc)
            fnc = big.tile([P, (t1 - t0) * W], FP8, tag=f"fnc{c}", name=f"fnc{c}")
            nc.sync.dma_start(fnc[:], fn_d[:, t0 * W : t1 * W])
            fn_chunks.append(fnc)

        def ft_tile(t):
            c, r = divmod(t, TPC)
            return ft_chunks[c][:, r * P : (r + 1) * P]

        def fn_tile(t):
            c, r = divmod(t, TPC)
            return fn_chunks[c][:, r * W : (r + 1) * W]

        H_all = big.tile([P, T * K], FP8)

        sums_pool = ctx.enter_context(tc.tile_pool(name="sumsps", bufs=1, space="PSUM"))
        sums_ps = [
            sums_pool.tile([P, W], F32, tag=f"sums{h}", name=f"sums{h}")
            for h in range(H2)
        ]

        # ---------------- pass 1 ----------------
        with (
            tc.tile_pool(name="p1", bufs=3) as p1,
            tc.tile_pool(name="p1ps", bufs=2, space="PSUM") as p1ps,
        ):
            for t in range(T):
                ftt = ft_tile(t)
                s2 = p1ps.tile([P, K], F32, tag="s2")
                nc.tensor.matmul(s2[:], ftt, ct2[:], start=True, stop=True)
                # score = c2 - 2*F@C^T; one-hot H written to the fp8 store
                score = p1.tile([P, K], F32, tag="score")
                rmin = p1.tile([P, 1], F32, tag="rmin")
                nc.vector.tensor_tensor(score[:], c2bc[:], s2[:], ALU.subtract)
                nc.vector.tensor_reduce(
                    rmin[:], score[:], mybir.AxisListType.X, ALU.min
                )
                Honehot = H_all[:, t * K : (t + 1) * K]
                nc.vector.tensor_scalar(Honehot, score[:], rmin[:], None, ALU.is_le)
                fnt = fn_tile(t)
                for h in range(H2):
                    nc.tensor.matmul(
                        sums_ps[h][:],
                        Honehot[:, h * P : (h + 1) * P],
                        fnt,
                        start=(t == 0),
                        stop=(t == T - 1),
                    )

        # ---------------- all-reduce sums/counts ----------------
        stats = big.tile([P, H2 * W], F32)
        for h in range(H2):
            nc.scalar.activation(stats[:, h * W : (h + 1) * W], sums_ps[h][:], ACTF.Copy)
        cc_in = dram.tile([P, H2 * W], F32)
        cc_out = dram.tile([P, H2 * W], F32)
        nc.sync.dma_start(cc_in[:], stats[:])
        nc.gpsimd.collective_compute(
            "AllReduce",
            ALU.add,
            replica_groups=[list(range(num_cores))],
            ins=[cc_in.opt()],
            outs=[cc_out.opt()],
        )
        gstats = big.tile([P, H2 * W], F32)
        nc.sync.dma_start(gstats[:], cc_out[:])

        # ---------------- means, -2*means^T (bf16), m2 row (bf16) ----------------
        mt2 = big.tile([P, K], BF16)
        m2row = big.tile([1, K], BF16)
        with (
            tc.tile_pool(name="mid", bufs=1) as mid,
            tc.tile_pool(name="midps", bufs=1, space="PSUM") as midps,
        ):
            for h in range(H2):
                cnt = gstats[:, h * W + D : h * W + D + 1]
                safe = mid.tile([P, 1], F32, tag=f"safe{h}")
                nc.vector.tensor_scalar_max(safe[:], cnt, 1.0)
                recip = mid.tile([P, 1], F32, tag=f"recip{h}")
                nc.vector.reciprocal(recip[:], safe[:])
                means_h = mid.tile([P, D], F32, tag=f"means{h}")
                nc.vector.tensor_scalar(
                    means_h[:], gstats[:, h * W : h * W + D], recip[:], None, ALU.mult
                )
                mt_ps = midps.tile([P, P], F32, tag=f"mtps{h}")
                nc.tensor.matmul(mt_ps[:], means_h[:], ident[:], start=True, stop=True)
                nc.scalar.activation(
                    mt2[:, h * P : (h + 1) * P], mt_ps[:], ACTF.Copy, scale=-2.0
                )
                m2h = mid.tile([P, 1], F32, tag=f"m2h{h}")
                junkm = mid.tile([P, D], F32, tag=f"junkm{h}")
                nc.scalar.activation(
                    junkm[:], means_h[:], ACTF.Square, accum_out=m2h[:]
                )
                m2_ps = midps.tile([1, P], F32, tag=f"m2ps{h}")
                nc.tensor.matmul(m2_ps[:], m2h[:], ident[:], start=True, stop=True)
                nc.scalar.activation(m2row[0:1, h * P : (h + 1) * P], m2_ps[:], ACTF.Copy)

        dev_pool = ctx.enter_context(tc.tile_pool(name="devps", bufs=1, space="PSUM"))
        dev_ps = [
            dev_pool.tile([P, 1], F32, tag=f"dev{h}", name=f"dev{h}")
            for h in range(H2)
        ]

        # ---------------- pass 2 ----------------
        with (
            tc.tile_pool(name="p2", bufs=3) as p2,
            tc.tile_pool(name="p2ps", bufs=2, space="PSUM") as p2ps,
        ):
            for t in range(T):
                ftt = ft_tile(t)
                g2 = p2ps.tile([P, K], F32, tag="g2")
                # g2 = -2*F@means^T + m2_j
                nc.tensor.matmul(g2[:], ftt, mt2[:], start=True, stop=False)
                nc.tensor.matmul(g2[:], onesrow[:], m2row[:], start=False, stop=True)
                Hh = H_all[:, t * K : (t + 1) * K]
                # dev_sq_i = rowsum((g2 + f2_i) .* H)   (rowsum(H) == 1)
                junk2 = p2.tile([P, K], BF16, tag="junk2")
                devsq = p2.tile([P, 1], F32, tag="devsq")
                nc.vector.scalar_tensor_tensor(
                    junk2[:], g2[:], f2_all[:, t : t + 1], Hh, ALU.add, ALU.mult,
                    accum_out=devsq[:],
                )
                devc = p2.tile([P, 1], F32, tag="devc")
                nc.vector.tensor_scalar_max(devc[:], devsq[:], 0.0)
                dev_col = p2.tile([P, 1], BF16, tag="dev_col")
                nc.scalar.activation(dev_col[:], devc[:], ACTF.Sqrt)
                for h in range(H2):
                    nc.tensor.matmul(
                        dev_ps[h][:],
                        Hh[:, h * P : (h + 1) * P],
                        dev_col[:],
                        start=(t == 0),
                        stop=(t == T - 1),
                    )

        # ---------------- all-reduce dev sums; emit ----------------
        devloc = big.tile([P, H2], F32)
        for h in range(H2):
            nc.vector.tensor_copy(devloc[:, h : h + 1], dev_ps[h][:])
        cc2_in = dram.tile([P, H2], F32)
        cc2_out = dram.tile([P, H2], F32)
        nc.sync.dma_start(cc2_in[:], devloc[:])
        nc.gpsimd.collective_compute(
            "AllReduce",
            ALU.add,
            replica_groups=[list(range(num_cores))],
            ins=[cc2_in.opt()],
            outs=[cc2_out.opt()],
        )
        devtot = big.tile([P, H2], F32)
        nc.sync.dma_start(devtot[:], cc2_out[:])

        outsb = big.tile([P, 4], F32)
        for h in range(H2):
            nc.vector.tensor_copy(outsb[:, h : h + 1], devtot[:, h : h + 1])
            nc.vector.tensor_copy(
                outsb[:, 2 + h : 3 + h], gstats[:, h * W + D : h * W + D + 1]
            )
        nc.sync.dma_start(out_d, outsb[:])

    nc.compile()
    return nc


def make_inputs(features, centers, T, num_cores):
    """Shard + preprocess the full inputs into per-core in_maps."""
    nloc = T * P
    W = D + 1
    bf16 = ml_dtypes.bfloat16
    f = np.ascontiguousarray(features, dtype=np.float32)
    c = np.ascontiguousarray(centers, dtype=np.float32)
    ct2 = np.ascontiguousarray((2.0 * c).T).astype(bf16)          # [D, K]
    c2 = np.sum(c.astype(np.float64) * c, axis=1).astype(np.float32)
    c2bc = np.broadcast_to(c2[None, :], (P, K)).copy()            # [P, K] f32
    ident = np.eye(P, dtype=np.float32)
    onesrow = np.ones((1, P), dtype=bf16)

    in_maps = []
    for i in range(num_cores):
        shard = f[i * nloc : (i + 1) * nloc]                      # [nloc, D]
        ft = np.ascontiguousarray(shard.T).astype(bf16)           # [D, nloc]
        # fn: per 128-row tile, natural layout with a ones column appended,
        # packed as [P, T*W]
        fr = shard.reshape(T, P, D)
        fn = np.concatenate(
            [fr, np.ones((T, P, 1), np.float32)], axis=2
        )  # [T, P, W]
        fn = np.ascontiguousarray(fn.transpose(1, 0, 2).reshape(P, T * W)).astype(
            ml_dtypes.float8_e4m3
        )
        f2 = (np.sum(shard.astype(np.float64) ** 2, axis=1) + EPS).astype(np.float32)
        f2 = np.ascontiguousarray(f2.reshape(T, P).T)             # [P, T]
        in_maps.append(
            {
                "ft": ft,
                "fn": fn,
                "f2": f2,
                "ct2": ct2,
                "c2bc": c2bc,
                "ident": ident,
                "onesrow": onesrow,
            }
        )
    return in_maps


def finish_host(out0, centers):
    """Final O(K) scalar reduction + tiny K x K inter-cluster term."""
    H2 = K // P
    devtot = np.concatenate([out0[:, h] for h in range(H2)])      # [K]
    counts = np.concatenate([out0[:, 2 + h] for h in range(H2)])  # [K]
    safe = np.maximum(counts, 1.0)
    per_cluster = devtot / safe
    valid = counts > 1.0
    n_valid = float(np.sum(valid.astype(np.float32)))
    if n_valid > 0:
        intra = float(np.sum(np.where(valid, per_cluster, 0.0)) / max(n_valid, 1.0))
    else:
        intra = 0.0

    c = centers.astype(np.float32)
    c2 = np.sum(c * c, axis=1)
    sq = c2[:, None] + c2[None, :] - 2.0 * (c @ c.T)
    cdist = np.sqrt(np.maximum(sq, 0.0))
    below = (np.triu(cdist, k=1) < MARGIN).astype(np.float32)
    inter = float(np.sum(below) / (K * (K - 1) / 2))

    return np.float32(inter + intra)


_CACHED = {}


def _get_program(T, num_cores):
    key = (T, num_cores)
    if key not in _CACHED:
        _CACHED[key] = build_program(T, num_cores)
    return _CACHED[key]


def run_on_hw(features, centers, T=N // NCORES // P, num_cores=NCORES, trace=False):
    nc = _get_program(T, num_cores)
    in_maps = make_inputs(features, centers, T, num_cores)
    res = run_bass_kernel_spmd(nc, in_maps, list(range(num_cores)), trace=trace)
    return res


def kernel(features, centers):
    res = run_on_hw(features, centers)
    out0 = res.results[0]["out"]
    return finish_host(out0, centers)


# revision 12
# speedup vs baseline: 1.1319x; 1.1319x over previous
"""ClusteringLoss (vq_codebook) Trainium2 kernel.

Data-parallel over N across 8 NeuronCores. Per core (N_loc = 32768):
  pass 1: s2 = 2*F@C^T (PE bf16, fp32 PSUM); score = c2 - s2 fused with the
          row-min reduce (DVE tensor_tensor_reduce); one-hot H = (score<=min);
          per-cluster [sums|counts] += H^T @ [F|1] (PE bf16, PSUM-accumulated)
  all-reduce [sums|counts] across the 8 cores (on-device collective)
  means = sums / max(counts,1) (fp32); mt2 = -2*means^T (bf16)
  pass 2: g2 = F@mt2 + m2_j (PE); gm_i = rowsum(H .* g2) (DVE fused);
          dev = sqrt(gm + ||f||^2) batched; per-cluster dev sums += H^T @ dev
  all-reduce dev sums; host does the final O(K) scalar reduction and the
  (tiny) K x K inter-cluster term.
"""

import numpy as np
import ml_dtypes

import concourse.bass as bass
import concourse.bacc as bacc
import concourse.tile as tile
from concourse import mybir
from concourse.bass_utils import run_bass_kernel_spmd

F32 = mybir.dt.float32
BF16 = mybir.dt.bfloat16
FP8 = mybir.dt.float8e4
ALU = mybir.AluOpType
ACTF = mybir.ActivationFunctionType

N, D, K = 262144, 128, 256
NCORES = 8
P = 128
MARGIN = 1.0
EPS = 0.01  # keeps sqrt's argument positive if a point coincides with its mean

# DMA chunk width (in 129-col fn tiles / 128-col ft tiles) per dma_start
TPC = 16  # tiles per DMA chunk

import os
_USE_TTR = os.environ.get("USE_TTR", "1") == "1"


def build_program(T, num_cores):
    """Build the SPMD Bass program. T = number of 128-row tiles per core."""
    nloc = T * P
    H2 = K // P  # number of 128-wide halves of the cluster axis (2)
    W = D + 1
    nc = bacc.Bacc(
        "TRN2",
        target_bir_lowering=False,
        debug=False,
        num_devices=num_cores,
    )

    ft_d = nc.dram_tensor("ft", [P, nloc], BF16, kind="ExternalInput").ap()
    fn_d = nc.dram_tensor("fn", [P, T * W], FP8, kind="ExternalInput").ap()
    f2_d = nc.dram_tensor("f2", [P, T], F32, kind="ExternalInput").ap()
    ct2_d = nc.dram_tensor("ct2", [P, K], BF16, kind="ExternalInput").ap()
    c2bc_d = nc.dram_tensor("c2bc", [P, K], F32, kind="ExternalInput").ap()
    ident_d = nc.dram_tensor("ident", [P, P], F32, kind="ExternalInput").ap()
    onesrow_d = nc.dram_tensor("onesrow", [1, P], BF16, kind="ExternalInput").ap()
    out_d = nc.dram_tensor("out", [P, 4], F32, kind="ExternalOutput").ap()
    outdev_d = nc.dram_tensor("outdev", [1, K], F32, kind="ExternalOutput").ap()

    from contextlib import ExitStack

    with tile.TileContext(nc) as tc, ExitStack() as ctx:
        const = ctx.enter_context(tc.tile_pool(name="const", bufs=1))
        big = ctx.enter_context(tc.tile_pool(name="big", bufs=1))
        dram = ctx.enter_context(tc.tile_pool(name="dram", bufs=1, space="DRAM"))

        ct2 = const.tile([P, K], BF16)
        nc.sync.dma_start(ct2[:], ct2_d)
        c2bc = const.tile([P, K], F32)
        nc.sync.dma_start(c2bc[:], c2bc_d)
        ident = const.tile([P, P], F32)
        nc.sync.dma_start(ident[:], ident_d)
        onesrow = const.tile([1, P], BF16)
        nc.sync.dma_start(onesrow[:], onesrow_d)
        f2_all = const.tile([P, T], F32)
        nc.sync.dma_start(f2_all[:], f2_d)

        # Resident feature shard, both layouts, loaded in chunks so compute
        # on early tiles overlaps later DMA.
        nchunks = (T + TPC - 1) // TPC
        ft_chunks, fn_chunks = [], []
        for c in range(nchunks):
            t0, t1 = c * TPC, min((c + 1) * TPC, T)
            ftc = big.tile([P, (t1 - t0) * P], BF16, tag=f"ftc{c}", name=f"ftc{c}")
            nc.sync.dma_start(ftc[:], ft_d[:, t0 * P : t1 * P])
            ft_chunks.append(ftc)
            fnc = big.tile([P, (t1 - t0) * W], FP8, tag=f"fnc{c}", name=f"fnc{c}")
            nc.sync.dma_start(fnc[:], fn_d[:, t0 * W : t1 * W])
            fn_chunks.append(fnc)

        def ft_tile(t):
            c, r = divmod(t, TPC)
            return ft_chunks[c][:, r * P : (r + 1) * P]

        def fn_tile(t):
            c, r = divmod(t, TPC)
            return fn_chunks[c][:, r * W : (r + 1) * W]

        H_all = big.tile([P, T * K], FP8)

        sums_pool = ctx.enter_context(tc.tile_pool(name="sumsps", bufs=1, space="PSUM"))
        sums_ps = [
            sums_pool.tile([P, W], F32, tag=f"sums{h}", name=f"sums{h}")
            for h in range(H2)
        ]

        # ---------------- pass 1 ----------------
        with (
            tc.tile_pool(name="p1", bufs=3) as p1,
            tc.tile_pool(name="p1ps", bufs=2, space="PSUM") as p1ps,
        ):
            for t in range(T):
                ftt = ft_tile(t)
                s2 = p1ps.tile([P, K], F32, tag="s2")
                nc.tensor.matmul(s2[:], ftt, ct2[:], start=True, stop=True)
                # score = c2 - 2*F@C^T; one-hot H written to the fp8 store
                score = p1.tile([P, K], F32, tag="score")
                rmin = p1.tile([P, 1], F32, tag="rmin")
                nc.vector.tensor_tensor(score[:], c2bc[:], s2[:], ALU.subtract)
                nc.vector.tensor_reduce(
                    rmin[:], score[:], mybir.AxisListType.X, ALU.min
                )
                Honehot = H_all[:, t * K : (t + 1) * K]
                nc.vector.tensor_scalar(Honehot, score[:], rmin[:], None, ALU.is_le)
                fnt = fn_tile(t)
                for h in range(H2):
                    nc.tensor.matmul(
                        sums_ps[h][:],
                        Honehot[:, h * P : (h + 1) * P],
                        fnt,
                        start=(t == 0),
                        stop=(t == T - 1),
                    )

        # ---------------- all-reduce sums/counts ----------------
        stats = big.tile([P, H2 * W], F32)
        for h in range(H2):
            nc.scalar.activation(stats[:, h * W : (h + 1) * W], sums_ps[h][:], ACTF.Copy)
        cc_in = dram.tile([P, H2 * W], F32)
        cc_out = dram.tile([P, H2 * W], F32)
        nc.sync.dma_start(cc_in[:], stats[:])
        nc.gpsimd.collective_compute(
            "AllReduce",
            ALU.add,
            replica_groups=[list(range(num_cores))],
            ins=[cc_in.opt()],
            outs=[cc_out.opt()],
        )
        gstats = big.tile([P, H2 * W], F32)
        nc.sync.dma_start(gstats[:], cc_out[:])

        # ---------------- means, -2*means^T (bf16), m2 row (bf16) ----------------
        mt2 = big.tile([P, K], BF16)
        m2row = big.tile([1, K], BF16)
        with (
            tc.tile_pool(name="mid", bufs=1) as mid,
            tc.tile_pool(name="midps", bufs=1, space="PSUM") as midps,
        ):
            for h in range(H2):
                cnt = gstats[:, h * W + D : h * W + D + 1]
                safe = mid.tile([P, 1], F32, tag=f"safe{h}")
                nc.vector.tensor_scalar_max(safe[:], cnt, 1.0)
                recip = mid.tile([P, 1], F32, tag=f"recip{h}")
                nc.vector.reciprocal(recip[:], safe[:])
                means_h = mid.tile([P, D], F32, tag=f"means{h}")
                nc.vector.tensor_scalar(
                    means_h[:], gstats[:, h * W : h * W + D], recip[:], None, ALU.mult
                )
                mt_ps = midps.tile([P, P], F32, tag=f"mtps{h}")
                nc.tensor.matmul(mt_ps[:], means_h[:], ident[:], start=True, stop=True)
                nc.scalar.activation(
                    mt2[:, h * P : (h + 1) * P], mt_ps[:], ACTF.Copy, scale=-2.0
                )
                m2h = mid.tile([P, 1], F32, tag=f"m2h{h}")
                junkm = mid.tile([P, D], F32, tag=f"junkm{h}")
                nc.scalar.activation(
                    junkm[:], means_h[:], ACTF.Square, accum_out=m2h[:]
                )
                m2_ps = midps.tile([1, P], F32, tag=f"m2ps{h}")
                nc.tensor.matmul(m2_ps[:], m2h[:], ident[:], start=True, stop=True)
                nc.scalar.activation(m2row[0:1, h * P : (h + 1) * P], m2_ps[:], ACTF.Copy)

        dev_pool = ctx.enter_context(tc.tile_pool(name="devps", bufs=1, space="PSUM"))
        dev_ps = dev_pool.tile([1, K], F32, tag="devps", name="devps")

        # ---------------- pass 2 ----------------
        with (
            tc.tile_pool(name="p2", bufs=3) as p2,
            tc.tile_pool(name="p2ps", bufs=2, space="PSUM") as p2ps,
        ):
            for t in range(T):
                ftt = ft_tile(t)
                g2 = p2ps.tile([P, K], F32, tag="g2")
                # g2 = -2*F@means^T + m2_j
                nc.tensor.matmul(g2[:], ftt, mt2[:], start=True, stop=False)
                nc.tensor.matmul(g2[:], onesrow[:], m2row[:], start=False, stop=True)
                Hh = H_all[:, t * K : (t + 1) * K]
                # dev_sq_i = rowsum((g2 + f2_i) .* H)   (rowsum(H) == 1)
                junk2 = p2.tile([P, K], F32, tag="junk2")
                devsq = p2.tile([P, 1], F32, tag="devsq")
                nc.vector.scalar_tensor_tensor(
                    junk2[:], g2[:], f2_all[:, t : t + 1], Hh, ALU.add, ALU.mult,
                    accum_out=devsq[:],
                )
                devc = p2.tile([P, 1], F32, tag="devc")
                nc.vector.tensor_scalar_max(devc[:], devsq[:], 0.0)
                dev_col = p2.tile([P, 1], FP8, tag="dev_col")
                nc.scalar.activation(dev_col[:], devc[:], ACTF.Sqrt)
                # devsum_j += sum_i H_ij * dev_i  (dev as the 1-col stationary)
                nc.tensor.matmul(
                    dev_ps[:], dev_col[:], Hh,
                    start=(t == 0), stop=(t == T - 1),
                )

        # ---------------- all-reduce dev sums; emit ----------------
        devloc = big.tile([1, K], F32)
        nc.vector.tensor_copy(devloc[:], dev_ps[:])
        cc2_in = dram.tile([1, K], F32)
        cc2_out = dram.tile([1, K], F32)
        nc.sync.dma_start(cc2_in[:], devloc[:])
        nc.gpsimd.collective_compute(
            "AllReduce",
            ALU.add,
            replica_groups=[list(range(num_cores))],
            ins=[cc2_in.opt()],
            outs=[cc2_out.opt()],
        )
        devtot = big.tile([1, K], F32)
        nc.sync.dma_start(devtot[:], cc2_out[:])

        outsb = big.tile([P, 4], F32)
        for h in range(H2):
            nc.vector.tensor_copy(outsb[:, 2 + h : 3 + h], gstats[:, h * W + D : h * W + D + 1])
        nc.sync.dma_start(out_d[:, 2:4], outsb[:, 2:4])
        nc.sync.dma_start(outdev_d, devtot[:])

    nc.compile()
    return nc


def make_inputs(features, centers, T, num_cores):
    """Shard + preprocess the full inputs into per-core in_maps."""
    nloc = T * P
    W = D + 1
    bf16 = ml_dtypes.bfloat16
    f = np.ascontiguousarray(features, dtype=np.float32)
    c = np.ascontiguousarray(centers, dtype=np.float32)
    ct2 = np.ascontiguousarray((2.0 * c).T).astype(bf16)          # [D, K]
    c2 = np.sum(c.astype(np.float64) * c, axis=1).astype(np.float32)
    c2bc = np.broadcast_to(c2[None, :], (P, K)).copy()            # [P, K] f32
    ident = np.eye(P, dtype=np.float32)
    onesrow = np.ones((1, P), dtype=bf16)

    in_maps = []
    for i in range(num_cores):
        shard = f[i * nloc : (i + 1) * nloc]                      # [nloc, D]
        ft = np.ascontiguousarray(shard.T).astype(bf16)           # [D, nloc]
        # fn: per 128-row tile, natural layout with a ones column appended,
        # packed as [P, T*W]
        fr = shard.reshape(T, P, D)
        fn = np.concatenate(
            [fr, np.ones((T, P, 1), np.float32)], axis=2
        )  # [T, P, W]
        fn = np.ascontiguousarray(fn.transpose(1, 0, 2).reshape(P, T * W)).astype(
            ml_dtypes.float8_e4m3
        )
        f2 = (np.sum(shard.astype(np.float64) ** 2, axis=1) + EPS).astype(np.float32)
        f2 = np.ascontiguousarray(f2.reshape(T, P).T)             # [P, T]
        in_maps.append(
            {
                "ft": ft,
                "fn": fn,
                "f2": f2,
                "ct2": ct2,
                "c2bc": c2bc,
                "ident": ident,
                "onesrow": onesrow,
            }
        )
    return in_maps


def finish_host(out0, outdev, centers):
    """Final O(K) scalar reduction + tiny K x K inter-cluster term."""
    H2 = K // P
    devtot = np.asarray(outdev).reshape(K).astype(np.float32)     # [K]
    counts = np.concatenate([out0[:, 2 + h] for h in range(H2)])  # [K]
    safe = np.maximum(counts, 1.0)
    per_cluster = devtot / safe
    valid = counts > 1.0
    n_valid = float(np.sum(valid.astype(np.float32)))
    if n_valid > 0:
        intra = float(np.sum(np.where(valid, per_cluster, 0.0)) / max(n_valid, 1.0))
    else:
        intra = 0.0

    c = centers.astype(np.float32)
    c2 = np.sum(c * c, axis=1)
    sq = c2[:, None] + c2[None, :] - 2.0 * (c @ c.T)
    cdist = np.sqrt(np.maximum(sq, 0.0))
    below = (np.triu(cdist, k=1) < MARGIN).astype(np.float32)
    inter = float(np.sum(below) / (K * (K - 1) / 2))

    return np.float32(inter + intra)


_CACHED = {}


def _get_program(T, num_cores):
    key = (T, num_cores)
    if key not in _CACHED:
        _CACHED[key] = build_program(T, num_cores)
    return _CACHED[key]


def run_on_hw(features, centers, T=N // NCORES // P, num_cores=NCORES, trace=False):
    nc = _get_program(T, num_cores)
    in_maps = make_inputs(features, centers, T, num_cores)
    res = run_bass_kernel_spmd(nc, in_maps, list(range(num_cores)), trace=trace)
    return res


def kernel(features, centers):
    res = run_on_hw(features, centers)
    r0 = res.results[0]
    return finish_host(r0["out"], r0["outdev"], centers)
